# revision 1
# baseline (speedup 1.0000x reference)
"""DeepseekV3 MLA attention kernel for 8 Trainium2 NeuronCores.

Sharding (tensor-parallel over heads + data-parallel over tokens):
  - Stage A (per core, its 256-token slice): latent = hidden @ Wkva first,
    rmsnorm + rope k_pe, AllGather it (AG_kv) while q_a = hidden @ Wqa still
    computes; q_a_n then AllGathered in two halves.  All feature-major.
  - Stage B3 (overlaps AG_qa): k_nope / v = Wkvb_c.T @ kv_a_n for this core's
    4 heads, v produced token-major directly.
  - Stage B1/B2: q = Wqb_c.T @ q_a_n, rope q_pe in place (swap via PE
    permutation matmul).
  - Stage B4: causal attention per head, scores kept transposed (k on
    partitions) so softmax reductions are PE ones-matmuls; no max-subtraction
    (scores verified small).  After each head: that head-slot's AllGather
    (bf16) runs while later heads compute, and the corresponding slice of the
    output projection accumulates into SBUF.
  - Stage C (interleaved): out_c = sum_j Wo_j.T @ attn_j with Wo in bf16,
    rows host-permuted to the per-head-slot gather order.

Projection/attention matmuls run in float32r (full-rate fp32); only the
attention *outputs* and Wo are bf16.  No on-device transposes: activations
stay feature-major; host pre-transposes the hidden slices and permutes weight
columns so GPT-J interleaved rope becomes two contiguous row-blocks.
"""

import math
import sys
import types

import numpy as np

# ---------------------------------------------------------------- constants
H = 32
D_NOPE = 128
D_ROPE = 64
D_QK = 192
D_V = 128
KV_LORA = 512
EPS = 1e-6
ROPE_THETA = 10000.0
FACTOR = 40.0
BETA_FAST, BETA_SLOW = 32.0, 1.0
ORIG_MAX_POS = 4096
MSCALE_ALL_DIM = 1.0

T = 2048
HID = 4096
QA = 1536  # q lora rank
NCORES = 8
HL = H // NCORES          # 4 heads per core
TSH = T // NCORES         # 256 tokens per core
QB_N = HL * D_QK          # 768 q columns per core
KB_N = HL * (D_NOPE + D_V)  # 1024 kv columns per core
WO_N = HID // NCORES      # 512 output columns per core
KVB = KV_LORA + D_ROPE    # 576
QAH = QA // 2             # 768, AG_qa half

_CACHE = {}


def _yarn_inv_freq():
    dim = D_ROPE
    pos_freqs = ROPE_THETA ** (np.arange(0, dim, 2, dtype=np.float64) / dim)
    inv_extra = 1.0 / pos_freqs
    inv_inter = 1.0 / (FACTOR * pos_freqs)

    def corr_dim(n_rot):
        return dim * math.log(ORIG_MAX_POS / (n_rot * 2 * math.pi)) / (2 * math.log(ROPE_THETA))

    low = max(math.floor(corr_dim(BETA_FAST)), 0)
    high = min(math.ceil(corr_dim(BETA_SLOW)), dim - 1)
    ramp = np.clip((np.arange(dim // 2, dtype=np.float64) - low) / max(high - low, 1e-3), 0, 1)
    inv_freq_mask = 1.0 - ramp
    inv_freq = inv_inter * (1 - inv_freq_mask) + inv_extra * inv_freq_mask
    return inv_freq.astype(np.float32)


def _install_ntff_hook():
    """Shim antenv.axon_hooks so run_bass_kernel_spmd(trace=True) can profile."""
    if "antenv.axon_hooks" in sys.modules:
        return
    mod = types.ModuleType("antenv.axon_hooks")
    mod._hook = None

    def set_axon_ntff_profile_hook(h):
        mod._hook = h

    def get_axon_ntff_profile_hook():
        return mod._hook

    mod.set_axon_ntff_profile_hook = set_axon_ntff_profile_hook
    mod.get_axon_ntff_profile_hook = get_axon_ntff_profile_hook
    sys.modules["antenv.axon_hooks"] = mod
    try:
        import antenv

        antenv.axon_hooks = mod
        from trn_agent_boot.trn_boot import _ntff_profile_via_ctypes

        hook = _ntff_profile_via_ctypes("/opt/axon/libaxon_pjrt.so")
        if hook is not None:
            set_axon_ntff_profile_hook(hook)
    except Exception:
        pass


# ---------------------------------------------------------------- program
def _build_program():
    if "nc" in _CACHE:
        return _CACHE["nc"]

    import concourse.bacc as bacc
    import concourse.tile as tile
    from concourse import mybir

    R = mybir.dt.float32r
    F = mybir.dt.float32
    B16 = mybir.dt.bfloat16
    AF = mybir.ActivationFunctionType

    nc = bacc.Bacc("TRN2", target_bir_lowering=False, debug=False, num_devices=NCORES)

    # ------------- DRAM I/O (per-core values fed via in_maps)
    hT = nc.dram_tensor("hT", [HID, TSH], B16, kind="ExternalInput")
    wqa = nc.dram_tensor("wqa", [HID, QA], B16, kind="ExternalInput")
    wkva = nc.dram_tensor("wkva", [HID, KVB], B16, kind="ExternalInput")
    wqb = nc.dram_tensor("wqb", [QA, QB_N], R, kind="ExternalInput")
    wkvb = nc.dram_tensor("wkvb", [KV_LORA, KB_N], R, kind="ExternalInput")
    wo = nc.dram_tensor("wo", [H * D_V, WO_N], B16, kind="ExternalInput")
    cca = nc.dram_tensor("cca", [D_ROPE // 2, TSH], F, kind="ExternalInput")
    ssa = nc.dram_tensor("ssa", [D_ROPE // 2, TSH], F, kind="ExternalInput")
    ccq = nc.dram_tensor("ccq", [128, T], F, kind="ExternalInput")
    ssq = nc.dram_tensor("ssq", [128, T], F, kind="ExternalInput")
    psw = nc.dram_tensor("psw", [128, 128], R, kind="ExternalInput")
    maskd = nc.dram_tensor("maskd", [128, 4 * 512], R, kind="ExternalInput")
    onesd = nc.dram_tensor("onesd", [128, 1], R, kind="ExternalInput")
    out = nc.dram_tensor("out", [WO_N, T], F, kind="ExternalOutput")

    NKH = HID // 128   # 32 hid chunks
    NKQ = QA // 128    # 12 q-lora chunks
    NKV = KV_LORA // 128  # 4 kv-lora chunks
    NB = 4             # token blocks of 512
    TB = 512
    RG = [list(range(NCORES))]

    with tile.TileContext(nc) as tc:
        with (
            tc.tile_pool(name="consts", bufs=1) as consts,
            tc.tile_pool(name="dram", bufs=1, space="DRAM") as dram,
        ):
            ones_sb = consts.tile([128, 1], R)
            nc.sync.dma_start(out=ones_sb[:], in_=onesd[:])
            eps_sb = consts.tile([1, 1], F)
            nc.vector.memset(eps_sb[:], EPS)

            ag_kv_in = dram.tile([KVB + 1, TSH], R)
            ag_kv_out = dram.tile([NCORES * (KVB + 1), TSH], R)
            ag_qa_in = [dram.tile([QAH + g, TSH], R, name=f"agqi{g}", tag=f"agqi{g}") for g in range(2)]
            ag_qa_out = [dram.tile([NCORES * (QAH + g), TSH], R, name=f"agqo{g}", tag=f"agqo{g}") for g in range(2)]
            ag2_in = [[dram.tile([D_V, T // 2], B16, name=f"ag2i{j}_{hf}", tag=f"ag2i{j}_{hf}")
                       for hf in range(2)] for j in range(HL)]
            ag2_out = [[dram.tile([NCORES * D_V, T // 2], B16, name=f"ag2o{j}_{hf}", tag=f"ag2o{j}_{hf}")
                        for hf in range(2)] for j in range(HL)]
            KVB1 = KVB + 1   # 577 rows per rank in ag_kv (576 + s_kv row)
            QAH1 = QAH + 1   # 769 rows per rank in ag_qa[1] (768 + s_q row)

            # ============================ Stage A
            # Ships RAW q_a / kv_a chunks as soon as they exit PSUM; the rms
            # scales travel as one extra AG row and are applied consumer-side.
            with (
                tc.tile_pool(name="a_ht", bufs=1) as a_ht,
                tc.tile_pool(name="a_stage", bufs=4) as a_stage,
                tc.tile_pool(name="a_small", bufs=1) as a_small,
            ):
                ht = []
                for k in range(NKH):
                    t_ = a_ht.tile([128, TSH], B16, name=f"ht{k}", tag=f"ht{k}")
                    nc.sync.dma_start(out=t_[:], in_=hT[k * 128:(k + 1) * 128, :])
                    ht.append(t_)

                with (
                    tc.tile_pool(name="a_w", bufs=3) as a_w,
                    tc.tile_pool(name="a_sq", bufs=3) as a_sq,
                    tc.tile_pool(name="a_ps", bufs=1, space="PSUM") as a_ps,
                ):
                    from concourse import bass_isa

                    # ---- kv path first (feeds AG_kv early)
                    with tc.tile_pool(name="a_pspe", bufs=1, space="PSUM") as a_pspe:
                        psk = [a_ps.tile([128, TSH], F, name=f"psk{m}", tag=f"psk{m}") for m in range(NKV)]
                        pspe = a_pspe.tile([D_ROPE, TSH], F, name="pspe", tag="pspe")
                        for k in range(NKH):
                            wband = a_w.tile([128, KVB], B16, name="wbandkv", tag="wbandkv")
                            nc.sync.dma_start(out=wband[:], in_=wkva[k * 128:(k + 1) * 128, :])
                            for m in range(NKV):
                                nc.tensor.matmul(
                                    psk[m][:], wband[:, m * 128:(m + 1) * 128], ht[k][:],
                                    start=(k == 0), stop=(k == NKH - 1))
                            nc.tensor.matmul(
                                pspe[:], wband[:, KV_LORA:KVB], ht[k][:],
                                start=(k == 0), stop=(k == NKH - 1))
                        kv_run = a_small.tile([128, TSH], F, name="kv_run", tag="kv_run")
                        for m in range(NKV):
                            st = a_stage.tile([128, TSH], R, name="kvst", tag="kvst")
                            nc.vector.tensor_copy(st[:], psk[m][:])
                            nc.scalar.dma_start(out=ag_kv_in[m * 128:(m + 1) * 128, :], in_=st[:])
                            sq = a_sq.tile([128, TSH], F, name="sq2", tag="sq2")
                            nc.scalar.activation(out=sq[:], in_=psk[m][:], func=AF.Square)
                            if m == 0:
                                nc.vector.tensor_copy(kv_run[:], sq[:])
                            else:
                                nc.vector.tensor_add(kv_run[:], kv_run[:], sq[:])
                        kv_tot = a_small.tile([128, TSH], F, name="kv_tot", tag="kv_tot")
                        nc.gpsimd.partition_all_reduce(
                            kv_tot[:], kv_run[:], 128, bass_isa.ReduceOp.add)

                        # rope k_pe (feature-major, grouped even/odd rows)
                        cca_sb = a_small.tile([D_ROPE // 2, TSH], F, name="cca", tag="cca")
                        ssa_sb = a_small.tile([D_ROPE // 2, TSH], F, name="ssa", tag="ssa")
                        nc.scalar.dma_start(out=cca_sb[:], in_=cca[:])
                        nc.scalar.dma_start(out=ssa_sb[:], in_=ssa[:])
                        HR = D_ROPE // 2
                        kpe_sb = a_small.tile([D_ROPE, TSH], R, name="kpe", tag="kpe")
                        t1 = a_small.tile([HR, TSH], F, name="t1", tag="t1")
                        t2 = a_small.tile([HR, TSH], F, name="t2", tag="t2")
                        nc.vector.tensor_mul(t1[:], pspe[0:HR, :], cca_sb[:])
                        nc.vector.tensor_mul(t2[:], pspe[HR:D_ROPE, :], ssa_sb[:])
                        nc.vector.tensor_sub(kpe_sb[0:HR, :], t1[:], t2[:])
                        t3 = a_small.tile([HR, TSH], F, name="t3", tag="t3")
                        t4 = a_small.tile([HR, TSH], F, name="t4", tag="t4")
                        nc.vector.tensor_mul(t3[:], pspe[HR:D_ROPE, :], cca_sb[:])
                        nc.vector.tensor_mul(t4[:], pspe[0:HR, :], ssa_sb[:])
                        nc.vector.tensor_add(kpe_sb[HR:D_ROPE, :], t3[:], t4[:])
                        nc.scalar.dma_start(out=ag_kv_in[KV_LORA:KVB, :], in_=kpe_sb[:])

                        skvr = a_small.tile([1, TSH], F, name="skvr", tag="skvr")
                        nc.scalar.activation(out=skvr[:], in_=kv_tot[0:1, :], func=AF.Sqrt,
                                             bias=eps_sb[:], scale=1.0 / KV_LORA)
                        rkv = a_small.tile([1, TSH], R, name="rkv", tag="rkv")
                        with nc.allow_low_precision(reason="fp32r rms scale, full fp32 mantissa-"):
                            nc.vector.reciprocal(out=rkv[:], in_=skvr[:])
                        nc.scalar.dma_start(out=ag_kv_in[KVB:KVB1, :], in_=rkv[:])

                    nc.gpsimd.collective_compute(
                        "AllGather", mybir.AluOpType.bypass, replica_groups=RG,
                        ins=[ag_kv_in.opt()], outs=[ag_kv_out.opt()])

                    # ---- q_a path: 4 M-groups of 3 chunks reusing the kv PSUM
                    # slots; raw chunks shipped immediately; first half gathers
                    # before the stats are done.
                    qa_run = a_small.tile([128, TSH], F, name="qa_run", tag="qa_run")
                    for mg in range(4):
                        psq = [a_ps.tile([128, TSH], F, name=f"psk{m}", tag=f"psk{m}") for m in range(3)]
                        for k in range(NKH):
                            wband = a_w.tile([128, 384], B16, name="wband", tag="wband")
                            nc.sync.dma_start(
                                out=wband[:],
                                in_=wqa[k * 128:(k + 1) * 128, mg * 384:(mg + 1) * 384])
                            for m in range(3):
                                nc.tensor.matmul(
                                    psq[m][:], wband[:, m * 128:(m + 1) * 128], ht[k][:],
                                    start=(k == 0), stop=(k == NKH - 1))
                        for m in range(3):
                            gm = mg * 3 + m
                            g, mm_ = divmod(gm, 6)
                            st = a_stage.tile([128, TSH], R, name="qst", tag="qst")
                            nc.vector.tensor_copy(st[:], psq[m][:])
                            nc.scalar.dma_start(
                                out=ag_qa_in[g][mm_ * 128:(mm_ + 1) * 128, :], in_=st[:])
                            sq = a_sq.tile([128, TSH], F, name="sq", tag="sq")
                            nc.scalar.activation(out=sq[:], in_=psq[m][:], func=AF.Square)
                            if gm == 0:
                                nc.vector.tensor_copy(qa_run[:], sq[:])
                            else:
                                nc.vector.tensor_add(qa_run[:], qa_run[:], sq[:])
                        if mg == 1:
                            nc.gpsimd.collective_compute(
                                "AllGather", mybir.AluOpType.bypass, replica_groups=RG,
                                ins=[ag_qa_in[0].opt()], outs=[ag_qa_out[0].opt()])
                    qa_tot = a_small.tile([128, TSH], F, name="qa_tot", tag="qa_tot")
                    nc.gpsimd.partition_all_reduce(
                        qa_tot[:], qa_run[:], 128, bass_isa.ReduceOp.add)
                    sqr = a_small.tile([1, TSH], F, name="sqr", tag="sqr")
                    nc.scalar.activation(out=sqr[:], in_=qa_tot[0:1, :], func=AF.Sqrt,
                                         bias=eps_sb[:], scale=1.0 / QA)
                    rq = a_small.tile([1, TSH], R, name="rq", tag="rq")
                    with nc.allow_low_precision(reason="fp32r rms scale, full fp32 mantissa-"):
                        nc.vector.reciprocal(out=rq[:], in_=sqr[:])
                    nc.scalar.dma_start(out=ag_qa_in[1][QAH:QAH1, :], in_=rq[:])
                    nc.gpsimd.collective_compute(
                        "AllGather", mybir.AluOpType.bypass, replica_groups=RG,
                        ins=[ag_qa_in[1].opt()], outs=[ag_qa_out[1].opt()])

            # ============================ Stage B3: k_nope / v (overlaps AG_qa)
            with tc.tile_pool(name="b_kv", bufs=1) as b_kv:
                kn = [b_kv.tile([128, T], R, name=f"kn{h}", tag=f"kn{h}") for h in range(HL)]
                vt = [b_kv.tile([128, TB], R, name=f"vt{i}", tag=f"vt{i}") for i in range(16)]
                # k_pe duplicated on both partition halves so both q_pe offsets
                # (0 for even heads, 64 for odd heads) have a matching lhsT
                kpe_f = b_kv.tile([128, T], R, name="kpef", tag="kpef")
                for r in range(NCORES):
                    for half in range(2):
                        nc.sync.dma_start(
                            out=kpe_f[64 * half:64 * half + 64, r * TSH:(r + 1) * TSH],
                            in_=ag_kv_out[r * KVB1 + KV_LORA:r * KVB1 + KVB, :])

                with (
                    tc.tile_pool(name="b3_w", bufs=1) as b3_w,
                    tc.tile_pool(name="b3_s", bufs=2) as b3_s,
                    tc.tile_pool(name="b3_ps", bufs=1, space="PSUM") as b3_ps,
                ):
                    wkb = []
                    for k in range(NKV):
                        t_ = b3_w.tile([128, KB_N], R, name=f"wkb{k}", tag=f"wkb{k}")
                        nc.sync.dma_start(out=t_[:], in_=wkvb[k * 128:(k + 1) * 128, :])
                        wkb.append(t_)

                    for nb in range(NB):
                        skv = b3_s.tile([1, TB], R, name="skv", tag="skv")
                        for half in range(2):
                            r = 2 * nb + half
                            nc.sync.dma_start(
                                out=skv[:, half * TSH:(half + 1) * TSH],
                                in_=ag_kv_out[r * KVB1 + KVB:r * KVB1 + KVB1, :])
                        skvb = b3_s.tile([128, TB], R, name="skvb", tag="skvb")
                        nc.gpsimd.partition_broadcast(skvb[:], skv[:])
                        kva = []
                        for k in range(NKV):
                            t_ = b3_s.tile([128, TB], R, name=f"kva{k}", tag=f"kva{k}")
                            for half in range(2):
                                r = 2 * nb + half
                                nc.sync.dma_start(
                                    out=t_[:, half * TSH:(half + 1) * TSH],
                                    in_=ag_kv_out[r * KVB1 + k * 128:r * KVB1 + (k + 1) * 128, :])
                            nc.vector.tensor_mul(t_[:], t_[:], skvb[:])
                            kva.append(t_)
                        psk = [b3_ps.tile([128, TB], F, name=f"b3k{m}", tag=f"b3k{m}") for m in range(HL)]
                        for k in range(NKV):
                            for m in range(HL):
                                nc.tensor.matmul(
                                    psk[m][:], wkb[k][:, m * 128:(m + 1) * 128], kva[k][:],
                                    start=(k == 0), stop=(k == NKV - 1))
                        for m in range(HL):
                            if m % 2 == 0:
                                nc.vector.tensor_copy(kn[m][:, nb * TB:(nb + 1) * TB], psk[m][:])
                            else:
                                nc.scalar.copy(out=kn[m][:, nb * TB:(nb + 1) * TB], in_=psk[m][:])
                        for tch in range(4):
                            psv = b3_ps.tile([128, TB], F, name="b3v", tag=f"b3k{tch}")
                            for k in range(NKV):
                                nc.tensor.matmul(
                                    psv[:], kva[k][:, tch * 128:(tch + 1) * 128],
                                    wkb[k][:, HL * D_NOPE:KB_N],
                                    start=(k == 0), stop=(k == NKV - 1))
                            nc.vector.tensor_copy(vt[nb * 4 + tch][:], psv[:])

                # ============================ Stage B1/B2: q projection + rope
                with tc.tile_pool(name="b_q", bufs=1) as b_q:
                    qn = [b_q.tile([128, T], R, name=f"qn{h}", tag=f"qn{h}") for h in range(HL)]
                    qpe_fin = [b_q.tile([128, T], R, name=f"qpf{t_}", tag=f"qpf{t_}") for t_ in range(2)]

                    with (
                        tc.tile_pool(name="b1_w", bufs=1) as b1_w,
                        tc.tile_pool(name="b1_s", bufs=3) as b1_s,
                        tc.tile_pool(name="b2_t", bufs=2) as b2_t,
                        tc.tile_pool(name="b1_ps", bufs=1, space="PSUM") as b1_ps,
                        tc.tile_pool(name="b2_ps", bufs=2, space="PSUM") as b2_ps,
                    ):
                        wq = []
                        for k in range(NKQ):
                            t_ = b1_w.tile([128, QB_N], R, name=f"wq{k}", tag=f"wq{k}")
                            nc.sync.dma_start(out=t_[:], in_=wqb[k * 128:(k + 1) * 128, :])
                            wq.append(t_)
                        psw_sb = b1_w.tile([128, 128], R, name="psw", tag="psw")
                        nc.sync.dma_start(out=psw_sb[:], in_=psw[:])

                        QAH1_ = QAH + 1
                        for nb in range(NB):
                            cs = slice(nb * TB, (nb + 1) * TB)
                            sqv = b1_s.tile([1, TB], R, name="sqv", tag="sqv")
                            for half in range(2):
                                r = 2 * nb + half
                                nc.sync.dma_start(
                                    out=sqv[:, half * TSH:(half + 1) * TSH],
                                    in_=ag_qa_out[1][r * QAH1_ + QAH:r * QAH1_ + QAH1_, :])
                            rqb = b1_s.tile([128, TB], R, name="rqb", tag="rqb")
                            nc.gpsimd.partition_broadcast(rqb[:], sqv[:])
                            ps6 = [b1_ps.tile([128, TB], F, name=f"b1p{m}", tag=f"b1p{m}") for m in range(6)]
                            for k in range(NKQ):
                                g, kk = divmod(k, 6)
                                stride = QAH + g
                                rqa = b1_s.tile([128, TB], R, name="rqa", tag="rqa")
                                for half in range(2):
                                    r = 2 * nb + half
                                    nc.sync.dma_start(
                                        out=rqa[:, half * TSH:(half + 1) * TSH],
                                        in_=ag_qa_out[g][r * stride + kk * 128:r * stride + (kk + 1) * 128, :])
                                for m in range(6):
                                    nc.tensor.matmul(
                                        ps6[m][:], wq[k][:, m * 128:(m + 1) * 128], rqa[:],
                                        start=(k == 0), stop=(k == NKQ - 1))
                            for m in range(HL):
                                nc.vector.tensor_mul(qn[m][:, cs], ps6[m][:], rqb[:])
                            # rope in place on the scaled q_pe
                            for t_ in range(2):
                                raw = ps6[4 + t_]
                                nc.vector.tensor_mul(qpe_fin[t_][:, cs], raw[:], rqb[:])
                                ps_sw = b2_ps.tile([128, TB], F, name="sw", tag="sw")
                                nc.tensor.matmul(ps_sw[:], psw_sb[:], qpe_fin[t_][:, cs],
                                                 start=True, stop=True)
                                ccs = b2_t.tile([128, TB], F, name="ccs", tag="ccs")
                                sss = b2_t.tile([128, TB], F, name="sss", tag="sss")
                                nc.sync.dma_start(out=ccs[:], in_=ccq[:, cs])
                                nc.sync.dma_start(out=sss[:], in_=ssq[:, cs])
                                tm1 = b2_t.tile([128, TB], F, name="tm1", tag="tm1")
                                tm2 = b2_t.tile([128, TB], F, name="tm2", tag="tm2")
                                nc.vector.tensor_mul(tm1[:], qpe_fin[t_][:, cs], ccs[:])
                                nc.vector.tensor_mul(tm2[:], ps_sw[:], sss[:])
                                nc.vector.tensor_add(qpe_fin[t_][:, cs], tm1[:], tm2[:])

                    # ============== Stage B4 + interleaved Stage C
                    with (
                        tc.tile_pool(name="b4_c", bufs=1) as b4_c,
                        tc.tile_pool(name="b4_e", bufs=3) as b4_e,
                        tc.tile_pool(name="b4_at", bufs=2) as b4_at,
                        tc.tile_pool(name="b4_sm", bufs=2) as b4_sm,
                        tc.tile_pool(name="c_acc", bufs=1) as c_acc,
                        tc.tile_pool(name="c_w", bufs=1) as c_w,
                        tc.tile_pool(name="c_s", bufs=1) as c_s,
                        tc.tile_pool(name="b4_ps", bufs=3, space="PSUM") as b4_ps,
                        tc.tile_pool(name="b4_po", bufs=2, space="PSUM") as b4_po,
                        tc.tile_pool(name="c_ps", bufs=2, space="PSUM") as c_ps,
                    ):
                        from concourse import bass_isa

                        mask_sb = b4_c.tile([128, 4 * 512], R, name="mask", tag="mask")
                        nc.sync.dma_start(out=mask_sb[:], in_=maskd[:])
                        acc = [c_acc.tile([128, TB], F, name=f"acc{i}", tag=f"acc{i}")
                               for i in range(16)]  # [nb*4 + mo]

                        for h in range(HL):
                            off = 64 * (h % 2)
                            qpe_h = qpe_fin[h // 2][off:off + 64, :]
                            kpe_h = kpe_f[off:off + 64, :]
                            j = h
                            wos = []
                            for r in range(NCORES):
                                t_ = c_w.tile([128, WO_N], B16, name=f"wos{r}", tag=f"wos{r}")
                                nc.sync.dma_start(
                                    out=t_[:],
                                    in_=wo[(j * NCORES + r) * 128:(j * NCORES + r + 1) * 128, :])
                                wos.append(t_)
                            for qj in (2, 3, 0, 1):
                                qs = slice(qj * TB, (qj + 1) * TB)
                                nki = 4 * qj + 4
                                ps_o = b4_po.tile([128, TB], F, name="pso", tag="pso")
                                exs = b4_sm.tile([128, TB], F, name="exs", tag="exs")
                                for ki in range(nki):
                                    ps_s = b4_ps.tile([128, TB], F, name="pss", tag="pss")
                                    nc.tensor.matmul(
                                        ps_s[:], kn[h][:, ki * 128:(ki + 1) * 128], qn[h][:, qs],
                                        start=True, stop=False)
                                    nc.tensor.matmul(
                                        ps_s[:], kpe_h[:, ki * 128:(ki + 1) * 128], qpe_h[:, qs],
                                        start=False, stop=True)
                                    ex = b4_e.tile([128, TB], R, name="ex", tag="ex")
                                    nc.scalar.activation(out=ex[:], in_=ps_s[:], func=AF.Exp)
                                    d = ki - 4 * qj
                                    if d >= 0:
                                        nc.vector.tensor_mul(
                                            ex[:], ex[:], mask_sb[:, d * 512:(d + 1) * 512])
                                    if ki == 0:
                                        nc.vector.tensor_copy(exs[:], ex[:])
                                    else:
                                        nc.vector.tensor_add(exs[:], exs[:], ex[:])
                                    nc.tensor.matmul(
                                        ps_o[:], vt[ki][:, h * 128:(h + 1) * 128], ex[:],
                                        start=(ki == 0), stop=(ki == nki - 1))
                                dsum = b4_sm.tile([128, TB], F, name="dsum", tag="dsum")
                                nc.gpsimd.partition_all_reduce(
                                    dsum[:], exs[:], 128, bass_isa.ReduceOp.add)
                                recb = b4_sm.tile([128, TB], F, name="recb", tag="recb")
                                nc.vector.reciprocal(out=recb[:], in_=dsum[:])
                                at = b4_at.tile([128, TB], B16, name="at", tag="at")
                                nc.vector.tensor_mul(at[:], ps_o[:], recb[:])
                                hf = qj // 2
                                nc.scalar.dma_start(
                                    out=ag2_in[h][hf][:, (qj % 2) * TB:(qj % 2 + 1) * TB], in_=at[:])
                                if qj % 2 == 1:
                                    # this half of the head slot is complete:
                                    # gather it and fold it into the output
                                    nc.gpsimd.collective_compute(
                                        "AllGather", mybir.AluOpType.bypass, replica_groups=RG,
                                        ins=[ag2_in[h][hf].opt()], outs=[ag2_out[h][hf].opt()])
                                    cs2 = slice(hf * 2 * TB, (hf + 1) * 2 * TB)
                                    rats = []
                                    for r in range(NCORES):
                                        t_ = c_s.tile([128, 2 * TB], B16, name=f"rat{r}", tag=f"rat{r}")
                                        nc.sync.dma_start(
                                            out=t_[:], in_=ag2_out[j][hf][r * 128:(r + 1) * 128, :])
                                        rats.append(t_)
                                    for nbq in range(2):
                                        nb_ = hf * 2 + nbq
                                        for mo in range(4):
                                            psc = c_ps.tile([128, TB], F, name="psc", tag="psc")
                                            for r in range(NCORES):
                                                nc.tensor.matmul(
                                                    psc[:],
                                                    wos[r][:, mo * 128:(mo + 1) * 128],
                                                    rats[r][:, nbq * TB:(nbq + 1) * TB],
                                                    start=(r == 0), stop=(r == NCORES - 1))
                                            a_ = acc[nb_ * 4 + mo]
                                            if j == 0:
                                                nc.vector.tensor_copy(a_[:], psc[:])
                                            elif j < HL - 1:
                                                nc.vector.tensor_add(a_[:], a_[:], psc[:])
                                            else:
                                                nc.vector.tensor_add(a_[:], a_[:], psc[:])
                                                nc.scalar.dma_start(
                                                    out=out[mo * 128:(mo + 1) * 128,
                                                            nb_ * TB:(nb_ + 1) * TB],
                                                    in_=a_[:])

    nc.compile()
    _CACHE["nc"] = nc
    return nc


# ---------------------------------------------------------------- host prep
def _prep_inputs(positions, hidden_states, Wqa, q_a_ln, Wqb, Wkva, kv_a_ln, Wkvb, Wo):
    import ml_dtypes

    positions = np.asarray(positions)
    hidden_states = np.ascontiguousarray(np.asarray(hidden_states, dtype=np.float32))
    Wqa = np.ascontiguousarray(np.asarray(Wqa, dtype=np.float32))
    q_a_ln = np.asarray(q_a_ln, dtype=np.float32)
    Wqb = np.asarray(Wqb, dtype=np.float32)
    Wkva = np.asarray(Wkva, dtype=np.float32)
    kv_a_ln = np.asarray(kv_a_ln, dtype=np.float32)
    Wkvb = np.asarray(Wkvb, dtype=np.float32)
    Wo = np.ascontiguousarray(np.asarray(Wo, dtype=np.float32))

    mscale = 0.1 * MSCALE_ALL_DIM * math.log(FACTOR) + 1.0
    scaling = (D_QK ** -0.5) * mscale * mscale

    inv_freq = _yarn_inv_freq()
    freqs = positions.astype(np.float32)[:, None] * inv_freq[None, :]  # [T, 32]
    cos = np.cos(freqs).astype(np.float32)
    sin = np.sin(freqs).astype(np.float32)

    HR = D_ROPE // 2
    perm = np.concatenate([np.arange(0, D_ROPE, 2), np.arange(1, D_ROPE, 2)])  # even|odd

    # Wqb: fold q_a_ln + scaling, permute per-core columns
    wqb_eff = (q_a_ln[:, None] * Wqb).reshape(QA, H, D_QK) * scaling
    wqb_cores = []
    for c in range(NCORES):
        hs = range(c * HL, (c + 1) * HL)
        cols = [wqb_eff[:, h_, :D_NOPE] for h_ in hs]
        cols += [wqb_eff[:, h_, D_NOPE + perm] for h_ in hs]
        wqb_cores.append(np.ascontiguousarray(np.concatenate(cols, axis=1), dtype=np.float32))

    # Wkva: rope perm on the k_pe columns
    wkva_p = Wkva.copy()
    wkva_p[:, KV_LORA:] = Wkva[:, KV_LORA + perm]
    wkva_p = np.ascontiguousarray(wkva_p, dtype=np.float32)
    wkva_b = np.ascontiguousarray(wkva_p.astype(ml_dtypes.bfloat16))
    wqa_b = np.ascontiguousarray(Wqa.astype(ml_dtypes.bfloat16))

    # Wkvb: fold kv_a_ln, per-core [k_nope x4 | v x4]
    wkvb_eff = (kv_a_ln[:, None] * Wkvb).reshape(KV_LORA, H, D_NOPE + D_V)
    wkvb_cores = []
    for c in range(NCORES):
        hs = range(c * HL, (c + 1) * HL)
        cols = [wkvb_eff[:, h_, :D_NOPE] for h_ in hs]
        cols += [wkvb_eff[:, h_, D_NOPE:] for h_ in hs]
        wkvb_cores.append(np.ascontiguousarray(np.concatenate(cols, axis=1), dtype=np.float32))

    # Wo rows permuted to the stage-C gather order: slot j, rank r -> head 4r+j
    row_order = []
    for j in range(HL):
        for r in range(NCORES):
            h_ = HL * r + j
            row_order.extend(range(h_ * D_V, (h_ + 1) * D_V))
    wo_p = Wo[row_order, :].astype(ml_dtypes.bfloat16)

    # rope ext tiles for q (2 heads per 128-row tile: [e,o | e,o] x 32 rows each)
    cosT = cos.T  # [32, T]
    sinT = sin.T
    ccq = np.ascontiguousarray(np.tile(cosT, (4, 1)), dtype=np.float32)      # [128, T]
    ssq = np.ascontiguousarray(np.concatenate([-sinT, sinT, -sinT, sinT], axis=0), dtype=np.float32)

    # swap permutation: within each 64-row block, rows 0:32 <-> 32:64
    pswm = np.zeros((128, 128), dtype=np.float32)
    for j in range(128):
        base = (j // 64) * 64
        off = j % 64
        k = base + (off + HR) % 64
        pswm[k, j] = 1.0

    # causal masks for the 4 diagonal offsets (512-wide q blocks, 128-wide k chunks)
    pos = positions.astype(np.int64)
    maskd = np.zeros((128, 4 * 512), dtype=np.float32)
    for d in range(4):
        kpos = pos[d * 128:(d + 1) * 128]   # relative within a q block
        qpos = pos[0:512]
        maskd[:, d * 512:(d + 1) * 512] = (kpos[:, None] <= qpos[None, :]).astype(np.float32)

    per_core = []
    for c in range(NCORES):
        sl = slice(c * TSH, (c + 1) * TSH)
        per_core.append({
            "hT": np.ascontiguousarray(hidden_states[sl].T.astype(ml_dtypes.bfloat16)),
            "wqa": wqa_b,
            "wkva": wkva_b,
            "wqb": wqb_cores[c],
            "wkvb": wkvb_cores[c],
            "wo": np.ascontiguousarray(wo_p[:, c * WO_N:(c + 1) * WO_N]),
            "cca": np.ascontiguousarray(cosT[:, sl]),
            "ssa": np.ascontiguousarray(sinT[:, sl]),
            "ccq": ccq,
            "ssq": ssq,
            "psw": pswm,
            "maskd": maskd,
            "onesd": np.ones((128, 1), dtype=np.float32),
        })
    return per_core


def run(inputs, trace=False):
    """Build + run; returns (full_output [T, HID] fp32, exec_time_ns or None)."""
    _install_ntff_hook()
    from concourse.bass_utils import run_bass_kernel_spmd

    nc = _build_program()
    in_maps = _prep_inputs(**inputs)
    res = run_bass_kernel_spmd(nc, in_maps, list(range(NCORES)), trace=trace)
    out = np.empty((T, HID), dtype=np.float32)
    for c in range(NCORES):
        out[:, c * WO_N:(c + 1) * WO_N] = res.results[c]["out"].T
    return out, res.exec_time_ns


def kernel(**inputs):
    out, _ = run(inputs, trace=False)
    return out



# revision 5
# speedup vs baseline: 1.3523x; 1.3523x over previous
"""DeepseekV3 MLA attention kernel for 8 Trainium2 NeuronCores.

Sharding (tensor-parallel over heads + data-parallel over tokens), all-bf16
dataflow (fp32 only in PSUM and softmax stats):

  - Stage A (per core, its 256-token slice): latent = hidden @ Wkva,
    producer-side rmsnorm (sum-of-squares via PE ones-matmul), rope k_pe,
    ship normalized bf16 kv_a; AllGather (Shared output) overlaps the q_a
    pass.  q_a likewise producer-normalized and gathered in bf16.
  - Stage B3 (overlaps AG_qa): k_nope / v = Wkvb_c.T @ kv_a_n for this core's
    4 heads, v token-major.  k_pe expanded into two zero-padded tiles
    (kpe_lo / kpe_hi) so B4's rope-score matmuls contract over full 128
    partitions.
  - Stage B1/B2: q = Wqb_c.T @ q_a_n, rope q_pe in place (swap via PE
    permutation matmul).
  - Stage B4: causal attention per head, scores transposed (k on partitions).
    exp on Scalar -> bf16; denominator via PE ones-matmul accumulated in
    PSUM; reciprocal on [1,512] + gpsimd partition broadcast.  After each
    head-half: bf16 AllGather (Shared out) of that head slot overlapping
    later compute, and the output projection accumulates into SBUF.
  - Stage C (interleaved): out_c = sum_j Wo_j.T @ attn_j, Wo bf16, rows
    host-permuted to the per-head-slot gather order.
"""

import math
import sys
import types

import numpy as np

# ---------------------------------------------------------------- constants
H = 32
D_NOPE = 128
D_ROPE = 64
D_QK = 192
D_V = 128
KV_LORA = 512
EPS = 1e-6
ROPE_THETA = 10000.0
FACTOR = 40.0
BETA_FAST, BETA_SLOW = 32.0, 1.0
ORIG_MAX_POS = 4096
MSCALE_ALL_DIM = 1.0

T = 2048
HID = 4096
QA = 1536  # q lora rank
NCORES = 8
HL = H // NCORES          # 4 heads per core
TSH = T // NCORES         # 256 tokens per core
QB_N = HL * D_QK          # 768 q columns per core
KB_N = HL * (D_NOPE + D_V)  # 1024 kv columns per core
WO_N = HID // NCORES      # 512 output columns per core
KVB = KV_LORA + D_ROPE    # 576

_CACHE = {}


def _yarn_inv_freq():
    dim = D_ROPE
    pos_freqs = ROPE_THETA ** (np.arange(0, dim, 2, dtype=np.float64) / dim)
    inv_extra = 1.0 / pos_freqs
    inv_inter = 1.0 / (FACTOR * pos_freqs)

    def corr_dim(n_rot):
        return dim * math.log(ORIG_MAX_POS / (n_rot * 2 * math.pi)) / (2 * math.log(ROPE_THETA))

    low = max(math.floor(corr_dim(BETA_FAST)), 0)
    high = min(math.ceil(corr_dim(BETA_SLOW)), dim - 1)
    ramp = np.clip((np.arange(dim // 2, dtype=np.float64) - low) / max(high - low, 1e-3), 0, 1)
    inv_freq_mask = 1.0 - ramp
    inv_freq = inv_inter * (1 - inv_freq_mask) + inv_extra * inv_freq_mask
    return inv_freq.astype(np.float32)


def _install_ntff_hook():
    """Shim antenv.axon_hooks so run_bass_kernel_spmd(trace=True) can profile."""
    if "antenv.axon_hooks" in sys.modules:
        return
    mod = types.ModuleType("antenv.axon_hooks")
    mod._hook = None

    def set_axon_ntff_profile_hook(h):
        mod._hook = h

    def get_axon_ntff_profile_hook():
        return mod._hook

    mod.set_axon_ntff_profile_hook = set_axon_ntff_profile_hook
    mod.get_axon_ntff_profile_hook = get_axon_ntff_profile_hook
    sys.modules["antenv.axon_hooks"] = mod
    try:
        import antenv

        antenv.axon_hooks = mod
        from trn_agent_boot.trn_boot import _ntff_profile_via_ctypes

        hook = _ntff_profile_via_ctypes("/opt/axon/libaxon_pjrt.so")
        if hook is not None:
            set_axon_ntff_profile_hook(hook)
    except Exception:
        pass


# ---------------------------------------------------------------- program
def _build_program():
    if "nc" in _CACHE:
        return _CACHE["nc"]

    import concourse.bacc as bacc
    import concourse.tile as tile
    from concourse import mybir

    F = mybir.dt.float32
    B16 = mybir.dt.bfloat16
    AF = mybir.ActivationFunctionType

    nc = bacc.Bacc("TRN2", target_bir_lowering=False, debug=False, num_devices=NCORES)

    # ------------- DRAM I/O (per-core values fed via in_maps)
    hT = nc.dram_tensor("hT", [HID, TSH], B16, kind="ExternalInput")
    wqa = nc.dram_tensor("wqa", [HID, QA], B16, kind="ExternalInput")
    wkva = nc.dram_tensor("wkva", [HID, KVB], B16, kind="ExternalInput")
    wqb = nc.dram_tensor("wqb", [QA, QB_N], B16, kind="ExternalInput")
    wkvb = nc.dram_tensor("wkvb", [KV_LORA, KB_N], B16, kind="ExternalInput")
    wo = nc.dram_tensor("wo", [H * D_V, WO_N], B16, kind="ExternalInput")
    cca = nc.dram_tensor("cca", [D_ROPE // 2, TSH], F, kind="ExternalInput")
    ssa = nc.dram_tensor("ssa", [D_ROPE // 2, TSH], F, kind="ExternalInput")
    ccq = nc.dram_tensor("ccq", [128, T], B16, kind="ExternalInput")
    ssq = nc.dram_tensor("ssq", [128, T], B16, kind="ExternalInput")
    psw = nc.dram_tensor("psw", [128, 128], B16, kind="ExternalInput")
    maskd = nc.dram_tensor("maskd", [128, 4 * 512], B16, kind="ExternalInput")
    onesd = nc.dram_tensor("onesd", [128, 1], B16, kind="ExternalInput")
    out = nc.dram_tensor("out", [WO_N, T], F, kind="ExternalOutput")

    NKH = HID // 128   # 32 hid chunks
    NKQ = QA // 128    # 12 q-lora chunks
    NKV = KV_LORA // 128  # 4 kv-lora chunks
    NB = 4             # token blocks of 512
    TB = 512
    HR = D_ROPE // 2
    RG = [list(range(NCORES))]

    with tile.TileContext(nc) as tc:
        with (
            tc.tile_pool(name="consts", bufs=1) as consts,
            tc.tile_pool(name="dram", bufs=1, space="DRAM") as dram,
        ):
            ones_sb = consts.tile([128, 1], B16)
            nc.sync.dma_start(out=ones_sb[:], in_=onesd[:])
            eps_sb = consts.tile([1, 1], F)
            nc.vector.memset(eps_sb[:], EPS)

            ag_kv_in = dram.tile([KVB, TSH], B16)
            ag_kv_out = dram.tile([NCORES * KVB, TSH], B16, addr_space="Shared")
            ag_qa_in = dram.tile([QB_N * 0 + QA, TSH], B16, name="agqi", tag="agqi")
            ag_qa_out = dram.tile([NCORES * QA, TSH], B16, name="agqo", tag="agqo",
                                  addr_space="Shared")
            ag2_in = [[dram.tile([D_V, T // 2], B16, name=f"ag2i{j}_{hf}", tag=f"ag2i{j}_{hf}")
                       for hf in range(2)] for j in range(HL)]
            ag2_out = [[dram.tile([NCORES * D_V, T // 2], B16, name=f"ag2o{j}_{hf}",
                                  tag=f"ag2o{j}_{hf}", addr_space="Shared")
                        for hf in range(2)] for j in range(HL)]

            # ============================ persistent B-stage tiles
            with (
                tc.tile_pool(name="b_kv", bufs=1) as b_kv,
                tc.tile_pool(name="b_q", bufs=1) as b_q,
                tc.tile_pool(name="b_w", bufs=1) as b_w,
            ):
                kn = [b_kv.tile([128, T], B16, name=f"kn{h}", tag=f"kn{h}") for h in range(HL)]
                vt = [b_kv.tile([128, TB], B16, name=f"vt{i}", tag=f"vt{i}") for i in range(16)]
                kpe_lo = b_kv.tile([128, T], B16, name="kpelo", tag="kpelo")
                kpe_hi = b_kv.tile([128, T], B16, name="kpehi", tag="kpehi")
                qn = [b_q.tile([128, T], B16, name=f"qn{h}", tag=f"qn{h}") for h in range(HL)]
                qpe_fin = [b_q.tile([128, T], B16, name=f"qpf{t_}", tag=f"qpf{t_}") for t_ in range(2)]

                # ============================ Stage A
                with (
                    tc.tile_pool(name="a_ht", bufs=1) as a_ht,
                    tc.tile_pool(name="a_stage", bufs=4) as a_stage,
                    tc.tile_pool(name="a_small", bufs=1) as a_small,
                ):
                    ht = []
                    for k in range(NKH):
                        t_ = a_ht.tile([128, TSH], B16, name=f"ht{k}", tag=f"ht{k}")
                        nc.sync.dma_start(out=t_[:], in_=hT[k * 128:(k + 1) * 128, :])
                        ht.append(t_)

                    # prefetch B-stage weights while stage A computes
                    wkb = []
                    for k in range(NKV):
                        t_ = b_w.tile([128, KB_N], B16, name=f"wkb{k}", tag=f"wkb{k}")
                        nc.sync.dma_start(out=t_[:], in_=wkvb[k * 128:(k + 1) * 128, :])
                        wkb.append(t_)
                    wq = []
                    for k in range(NKQ):
                        t_ = b_w.tile([128, QB_N], B16, name=f"wq{k}", tag=f"wq{k}")
                        nc.sync.dma_start(out=t_[:], in_=wqb[k * 128:(k + 1) * 128, :])
                        wq.append(t_)
                    psw_sb = b_w.tile([128, 128], B16, name="psw", tag="psw")
                    nc.sync.dma_start(out=psw_sb[:], in_=psw[:])
                    mask_sb = b_w.tile([128, 4 * 512], B16, name="mask", tag="mask")
                    nc.sync.dma_start(out=mask_sb[:], in_=maskd[:])

                    with (
                        tc.tile_pool(name="a_w", bufs=4) as a_w,
                        tc.tile_pool(name="a_sq", bufs=3) as a_sq,
                        tc.tile_pool(name="a_ps", bufs=1, space="PSUM") as a_ps,
                        tc.tile_pool(name="a_pst", bufs=1, space="PSUM") as a_pst,
                    ):
                        # ---- kv pass (feeds AG_kv early)
                        # PSUM tags aps0..aps5 are shared between the kv pass
                        # (4 lora chunks + rope) and the q waves (6 chunks)
                        psk = [a_ps.tile([128, TSH], F, name=f"psk{m}", tag=f"aps{m}")
                               for m in range(NKV)]
                        pspe = a_ps.tile([D_ROPE, TSH], F, name="pspe", tag="aps4")
                        for k in range(NKH):
                            wband = a_w.tile([128, KVB], B16, name="wbandkv", tag="wbandkv")
                            nc.sync.dma_start(out=wband[:], in_=wkva[k * 128:(k + 1) * 128, :])
                            for m in range(NKV):
                                nc.tensor.matmul(
                                    psk[m][:], wband[:, m * 128:(m + 1) * 128], ht[k][:],
                                    start=(k == 0), stop=(k == NKH - 1))
                            nc.tensor.matmul(
                                pspe[:], wband[:, KV_LORA:KVB], ht[k][:],
                                start=(k == 0), stop=(k == NKH - 1))
                        # sum of squares via PE ones-matmul
                        ssq_kv = a_pst.tile([1, TSH], F, name="ssqkv", tag="ssqkv")
                        sqs = []
                        for m in range(NKV):
                            sq = a_sq.tile([128, TSH], B16, name="sq2", tag="sq2")
                            nc.scalar.activation(out=sq[:], in_=psk[m][:], func=AF.Square)
                            nc.tensor.matmul(ssq_kv[:], ones_sb[:], sq[:],
                                             start=(m == 0), stop=(m == NKV - 1))
                            sqs.append(sq)
                        skvr = a_small.tile([1, TSH], F, name="skvr", tag="skvr")
                        nc.scalar.activation(out=skvr[:], in_=ssq_kv[:], func=AF.Sqrt,
                                             bias=eps_sb[:], scale=1.0 / KV_LORA)
                        rkv = a_small.tile([1, TSH], F, name="rkv", tag="rkv")
                        with nc.allow_low_precision(reason="rms scale reciprocal"):
                            nc.vector.reciprocal(out=rkv[:], in_=skvr[:])
                        rkvb = a_small.tile([128, TSH], F, name="rkvb", tag="rkvb")
                        nc.gpsimd.partition_broadcast(rkvb[:], rkv[:])
                        for m in range(NKV):
                            st = a_stage.tile([128, TSH], B16, name="kvst", tag="kvst")
                            nc.vector.tensor_mul(st[:], psk[m][:], rkvb[:])
                            nc.scalar.dma_start(out=ag_kv_in[m * 128:(m + 1) * 128, :], in_=st[:])

                        # rope k_pe (feature-major, grouped even/odd rows)
                        cca_sb = a_small.tile([HR, TSH], F, name="cca", tag="cca")
                        ssa_sb = a_small.tile([HR, TSH], F, name="ssa", tag="ssa")
                        nc.scalar.dma_start(out=cca_sb[:], in_=cca[:])
                        nc.scalar.dma_start(out=ssa_sb[:], in_=ssa[:])
                        kpe_sb = a_small.tile([D_ROPE, TSH], B16, name="kpe", tag="kpe")
                        t1 = a_small.tile([HR, TSH], F, name="t1", tag="t1")
                        t2 = a_small.tile([HR, TSH], F, name="t2", tag="t2")
                        nc.vector.tensor_mul(t1[:], pspe[0:HR, :], cca_sb[:])
                        nc.vector.tensor_mul(t2[:], pspe[HR:D_ROPE, :], ssa_sb[:])
                        nc.vector.tensor_sub(kpe_sb[0:HR, :], t1[:], t2[:])
                        t3 = a_small.tile([HR, TSH], F, name="t3", tag="t3")
                        t4 = a_small.tile([HR, TSH], F, name="t4", tag="t4")
                        nc.vector.tensor_mul(t3[:], pspe[HR:D_ROPE, :], cca_sb[:])
                        nc.vector.tensor_mul(t4[:], pspe[0:HR, :], ssa_sb[:])
                        nc.vector.tensor_add(kpe_sb[HR:D_ROPE, :], t3[:], t4[:])
                        nc.scalar.dma_start(out=ag_kv_in[KV_LORA:KVB, :], in_=kpe_sb[:])

                        nc.gpsimd.collective_compute(
                            "AllGather", mybir.AluOpType.bypass, replica_groups=RG,
                            ins=[ag_kv_in.opt()], outs=[ag_kv_out.opt()])

                        # ---- q_a pass: two waves of 6 chunks; raw chunks are
                        # cast to bf16 SBUF so the PSUM banks recycle per wave
                        ssq_q = a_pst.tile([1, TSH], F, name="ssqq", tag="ssqq")
                        qraw = [a_small.tile([128, TSH], B16, name=f"qraw{g}", tag=f"qraw{g}")
                                for g in range(NKQ)]
                        for wv in range(2):
                            psq = [a_ps.tile([128, TSH], F, name=f"psq{m}", tag=f"aps{m}")
                                   for m in range(6)]
                            for k in range(NKH):
                                wband = a_w.tile([128, 768], B16, name="wband", tag="wband")
                                nc.sync.dma_start(
                                    out=wband[:],
                                    in_=wqa[k * 128:(k + 1) * 128, wv * 768:(wv + 1) * 768])
                                for m in range(6):
                                    nc.tensor.matmul(
                                        psq[m][:], wband[:, m * 128:(m + 1) * 128], ht[k][:],
                                        start=(k == 0), stop=(k == NKH - 1))
                            for m in range(6):
                                g = wv * 6 + m
                                sq = a_sq.tile([128, TSH], B16, name="sq", tag="sq")
                                nc.scalar.activation(out=sq[:], in_=psq[m][:], func=AF.Square)
                                nc.tensor.matmul(ssq_q[:], ones_sb[:], sq[:],
                                                 start=(g == 0), stop=(g == NKQ - 1))
                                nc.vector.tensor_copy(qraw[g][:], psq[m][:])
                        sqr = a_small.tile([1, TSH], F, name="sqr", tag="sqr")
                        nc.scalar.activation(out=sqr[:], in_=ssq_q[:], func=AF.Sqrt,
                                             bias=eps_sb[:], scale=1.0 / QA)
                        rq = a_small.tile([1, TSH], F, name="rq", tag="rq")
                        with nc.allow_low_precision(reason="rms scale reciprocal"):
                            nc.vector.reciprocal(out=rq[:], in_=sqr[:])
                        rqb = a_small.tile([128, TSH], F, name="rqb", tag="rqb")
                        nc.gpsimd.partition_broadcast(rqb[:], rq[:])
                        for g in range(NKQ):
                            st = a_stage.tile([128, TSH], B16, name="qst", tag="qst")
                            nc.vector.tensor_mul(st[:], qraw[g][:], rqb[:])
                            nc.scalar.dma_start(
                                out=ag_qa_in[g * 128:(g + 1) * 128, :], in_=st[:])
                        nc.gpsimd.collective_compute(
                            "AllGather", mybir.AluOpType.bypass, replica_groups=RG,
                            ins=[ag_qa_in.opt()], outs=[ag_qa_out.opt()])

                # ============================ Stage B3: k_nope / v (overlaps AG_qa)
                with (
                    tc.tile_pool(name="b3_s", bufs=3) as b3_s,
                    tc.tile_pool(name="b3_ps", bufs=1, space="PSUM") as b3_ps,
                ):
                    # k_pe into zero-padded lo/hi tiles
                    nc.vector.memset(kpe_lo[:], 0.0)
                    nc.vector.memset(kpe_hi[:], 0.0)
                    for r in range(NCORES):
                        nc.sync.dma_start(
                            out=kpe_lo[0:D_ROPE, r * TSH:(r + 1) * TSH],
                            in_=ag_kv_out[r * KVB + KV_LORA:r * KVB + KVB, :])
                        nc.sync.dma_start(
                            out=kpe_hi[D_ROPE:128, r * TSH:(r + 1) * TSH],
                            in_=ag_kv_out[r * KVB + KV_LORA:r * KVB + KVB, :])

                    for nb in range(NB):
                        kva = []
                        for k in range(NKV):
                            t_ = b3_s.tile([128, TB], B16, name=f"kva{k}", tag=f"kva{k}")
                            for half in range(2):
                                r = 2 * nb + half
                                nc.sync.dma_start(
                                    out=t_[:, half * TSH:(half + 1) * TSH],
                                    in_=ag_kv_out[r * KVB + k * 128:r * KVB + (k + 1) * 128, :])
                            kva.append(t_)
                        psk3 = [b3_ps.tile([128, TB], F, name=f"b3k{m}", tag=f"b3k{m}")
                                for m in range(HL)]
                        for k in range(NKV):
                            for m in range(HL):
                                nc.tensor.matmul(
                                    psk3[m][:], wkb[k][:, m * 128:(m + 1) * 128], kva[k][:],
                                    start=(k == 0), stop=(k == NKV - 1))
                        psv = [b3_ps.tile([128, TB], F, name=f"b3v{tch}", tag=f"b3v{tch}")
                               for tch in range(4)]
                        for k in range(NKV):
                            for tch in range(4):
                                nc.tensor.matmul(
                                    psv[tch][:], kva[k][:, tch * 128:(tch + 1) * 128],
                                    wkb[k][:, HL * D_NOPE:KB_N],
                                    start=(k == 0), stop=(k == NKV - 1))
                        for m in range(HL):
                            if m % 2 == 0:
                                nc.vector.tensor_copy(kn[m][:, nb * TB:(nb + 1) * TB], psk3[m][:])
                            else:
                                nc.scalar.copy(out=kn[m][:, nb * TB:(nb + 1) * TB], in_=psk3[m][:])
                        for tch in range(4):
                            if tch % 2 == 0:
                                nc.vector.tensor_copy(vt[nb * 4 + tch][:], psv[tch][:])
                            else:
                                nc.scalar.copy(out=vt[nb * 4 + tch][:], in_=psv[tch][:])

                # ============================ Stage B1/B2: q projection + rope
                with (
                    tc.tile_pool(name="b1_s", bufs=3) as b1_s,
                    tc.tile_pool(name="b2_t", bufs=2) as b2_t,
                    tc.tile_pool(name="b1_ps", bufs=1, space="PSUM") as b1_ps,
                    tc.tile_pool(name="b2_ps", bufs=2, space="PSUM") as b2_ps,
                ):
                    for nb in range(NB):
                        cs = slice(nb * TB, (nb + 1) * TB)
                        ps6 = [b1_ps.tile([128, TB], F, name=f"b1p{m}", tag=f"b1p{m}")
                               for m in range(6)]
                        for k in range(NKQ):
                            rqa = b1_s.tile([128, TB], B16, name="rqa", tag="rqa")
                            for half in range(2):
                                r = 2 * nb + half
                                nc.sync.dma_start(
                                    out=rqa[:, half * TSH:(half + 1) * TSH],
                                    in_=ag_qa_out[r * QA + k * 128:r * QA + (k + 1) * 128, :])
                            for m in range(6):
                                nc.tensor.matmul(
                                    ps6[m][:], wq[k][:, m * 128:(m + 1) * 128], rqa[:],
                                    start=(k == 0), stop=(k == NKQ - 1))
                        for m in range(HL):
                            if m % 2 == 0:
                                nc.vector.tensor_copy(qn[m][:, cs], ps6[m][:])
                            else:
                                nc.scalar.copy(out=qn[m][:, cs], in_=ps6[m][:])
                        # rope in place on q_pe
                        for t_ in range(2):
                            nc.vector.tensor_copy(qpe_fin[t_][:, cs], ps6[4 + t_][:])
                            ps_sw = b2_ps.tile([128, TB], F, name="sw", tag="sw")
                            nc.tensor.matmul(ps_sw[:], psw_sb[:], qpe_fin[t_][:, cs],
                                             start=True, stop=True)
                            ccs = b2_t.tile([128, TB], B16, name="ccs", tag="ccs")
                            sss = b2_t.tile([128, TB], B16, name="sss", tag="sss")
                            nc.sync.dma_start(out=ccs[:], in_=ccq[:, cs])
                            nc.sync.dma_start(out=sss[:], in_=ssq[:, cs])
                            tm1 = b2_t.tile([128, TB], F, name="tm1", tag="tm1")
                            tm2 = b2_t.tile([128, TB], F, name="tm2", tag="tm2")
                            nc.vector.tensor_mul(tm1[:], qpe_fin[t_][:, cs], ccs[:])
                            nc.vector.tensor_mul(tm2[:], ps_sw[:], sss[:])
                            nc.vector.tensor_add(qpe_fin[t_][:, cs], tm1[:], tm2[:])

                # ============== Stage B4 + interleaved Stage C
                with (
                    tc.tile_pool(name="b4_e", bufs=3) as b4_e,
                    tc.tile_pool(name="b4_at", bufs=2) as b4_at,
                    tc.tile_pool(name="b4_sm", bufs=2) as b4_sm,
                    tc.tile_pool(name="c_acc", bufs=1) as c_acc,
                    tc.tile_pool(name="c_w", bufs=1) as c_w,
                    tc.tile_pool(name="c_s", bufs=1) as c_s,
                    tc.tile_pool(name="b4_ps", bufs=2, space="PSUM") as b4_ps,
                    tc.tile_pool(name="b4_po", bufs=2, space="PSUM") as b4_po,
                    tc.tile_pool(name="b4_pd", bufs=1, space="PSUM") as b4_pd,
                    tc.tile_pool(name="c_ps", bufs=1, space="PSUM") as c_ps,
                ):
                    acc = [c_acc.tile([128, TB], F, name=f"acc{i}", tag=f"acc{i}")
                           for i in range(16)]  # [nb*4 + mo]

                    for h in range(HL):
                        kpe_h = kpe_lo if h % 2 == 0 else kpe_hi
                        qpe_h = qpe_fin[h // 2]
                        j = h
                        wos = []
                        for r in range(NCORES):
                            t_ = c_w.tile([128, WO_N], B16, name=f"wos{r}", tag=f"wos{r}")
                            nc.sync.dma_start(
                                out=t_[:],
                                in_=wo[(j * NCORES + r) * 128:(j * NCORES + r + 1) * 128, :])
                            wos.append(t_)
                        for qj in (2, 3, 0, 1):
                            qs = slice(qj * TB, (qj + 1) * TB)
                            nki = 4 * qj + 4
                            ps_o = b4_po.tile([128, TB], F, name="pso", tag="pso")
                            dsum = b4_pd.tile([1, TB], F, name="dsum", tag="dsum")
                            for ki in range(nki):
                                ps_s = b4_ps.tile([128, TB], F, name="pss", tag="pss")
                                nc.tensor.matmul(
                                    ps_s[:], kn[h][:, ki * 128:(ki + 1) * 128], qn[h][:, qs],
                                    start=True, stop=False)
                                nc.tensor.matmul(
                                    ps_s[:], kpe_h[:, ki * 128:(ki + 1) * 128], qpe_h[:, qs],
                                    start=False, stop=True)
                                ex = b4_e.tile([128, TB], B16, name="ex", tag="ex")
                                nc.scalar.activation(out=ex[:], in_=ps_s[:], func=AF.Exp)
                                d = ki - 4 * qj
                                if d >= 0:
                                    nc.vector.tensor_mul(
                                        ex[:], ex[:], mask_sb[:, d * 512:(d + 1) * 512])
                                nc.tensor.matmul(
                                    dsum[:], ones_sb[:], ex[:],
                                    start=(ki == 0), stop=(ki == nki - 1))
                                nc.tensor.matmul(
                                    ps_o[:], vt[ki][:, h * 128:(h + 1) * 128], ex[:],
                                    start=(ki == 0), stop=(ki == nki - 1))
                            rec = b4_sm.tile([1, TB], F, name="rec", tag="rec")
                            nc.vector.reciprocal(out=rec[:], in_=dsum[:])
                            recb = b4_sm.tile([128, TB], F, name="recb", tag="recb")
                            nc.gpsimd.partition_broadcast(recb[:], rec[:])
                            at = b4_at.tile([128, TB], B16, name="at", tag="at")
                            nc.vector.tensor_mul(at[:], ps_o[:], recb[:])
                            hf = qj // 2
                            nc.scalar.dma_start(
                                out=ag2_in[h][hf][:, (qj % 2) * TB:(qj % 2 + 1) * TB], in_=at[:])
                            if qj % 2 == 1:
                                # this half of the head slot is complete:
                                # gather it and fold it into the output
                                nc.gpsimd.collective_compute(
                                    "AllGather", mybir.AluOpType.bypass, replica_groups=RG,
                                    ins=[ag2_in[h][hf].opt()], outs=[ag2_out[h][hf].opt()])
                                rats = []
                                for r in range(NCORES):
                                    t_ = c_s.tile([128, 2 * TB], B16, name=f"rat{r}", tag=f"rat{r}")
                                    nc.sync.dma_start(
                                        out=t_[:], in_=ag2_out[j][hf][r * 128:(r + 1) * 128, :])
                                    rats.append(t_)
                                for mo in range(4):
                                    psc = [c_ps.tile([128, TB], F, name=f"psc{q_}", tag=f"psc{q_}")
                                           for q_ in range(2)]
                                    for r in range(NCORES):
                                        for nbq in range(2):
                                            nc.tensor.matmul(
                                                psc[nbq][:],
                                                wos[r][:, mo * 128:(mo + 1) * 128],
                                                rats[r][:, nbq * TB:(nbq + 1) * TB],
                                                start=(r == 0), stop=(r == NCORES - 1))
                                    for nbq in range(2):
                                        nb_ = hf * 2 + nbq
                                        a_ = acc[nb_ * 4 + mo]
                                        if j == 0:
                                            nc.vector.tensor_copy(a_[:], psc[nbq][:])
                                        elif j < HL - 1:
                                            nc.vector.tensor_add(a_[:], a_[:], psc[nbq][:])
                                        else:
                                            nc.vector.tensor_add(a_[:], a_[:], psc[nbq][:])
                                            nc.scalar.dma_start(
                                                out=out[mo * 128:(mo + 1) * 128,
                                                        nb_ * TB:(nb_ + 1) * TB],
                                                in_=a_[:])

    nc.compile()
    _CACHE["nc"] = nc
    return nc


# ---------------------------------------------------------------- host prep
def _prep_inputs(positions, hidden_states, Wqa, q_a_ln, Wqb, Wkva, kv_a_ln, Wkvb, Wo):
    import ml_dtypes

    BF = ml_dtypes.bfloat16
    positions = np.asarray(positions)
    hidden_states = np.ascontiguousarray(np.asarray(hidden_states, dtype=np.float32))
    Wqa = np.ascontiguousarray(np.asarray(Wqa, dtype=np.float32))
    q_a_ln = np.asarray(q_a_ln, dtype=np.float32)
    Wqb = np.asarray(Wqb, dtype=np.float32)
    Wkva = np.asarray(Wkva, dtype=np.float32)
    kv_a_ln = np.asarray(kv_a_ln, dtype=np.float32)
    Wkvb = np.asarray(Wkvb, dtype=np.float32)
    Wo = np.ascontiguousarray(np.asarray(Wo, dtype=np.float32))

    mscale = 0.1 * MSCALE_ALL_DIM * math.log(FACTOR) + 1.0
    scaling = (D_QK ** -0.5) * mscale * mscale

    inv_freq = _yarn_inv_freq()
    freqs = positions.astype(np.float32)[:, None] * inv_freq[None, :]  # [T, 32]
    cos = np.cos(freqs).astype(np.float32)
    sin = np.sin(freqs).astype(np.float32)

    HR = D_ROPE // 2
    perm = np.concatenate([np.arange(0, D_ROPE, 2), np.arange(1, D_ROPE, 2)])  # even|odd

    # Wqb: fold q_a_ln + scaling, permute per-core columns
    wqb_eff = (q_a_ln[:, None] * Wqb).reshape(QA, H, D_QK) * scaling
    wqb_cores = []
    for c in range(NCORES):
        hs = range(c * HL, (c + 1) * HL)
        cols = [wqb_eff[:, h_, :D_NOPE] for h_ in hs]
        cols += [wqb_eff[:, h_, D_NOPE + perm] for h_ in hs]
        wqb_cores.append(np.ascontiguousarray(
            np.concatenate(cols, axis=1).astype(BF)))

    # Wkva: rope perm on the k_pe columns
    wkva_p = Wkva.copy()
    wkva_p[:, KV_LORA:] = Wkva[:, KV_LORA + perm]
    wkva_b = np.ascontiguousarray(wkva_p.astype(BF))
    wqa_b = np.ascontiguousarray(Wqa.astype(BF))

    # Wkvb: fold kv_a_ln, per-core [k_nope x4 | v x4]
    wkvb_eff = (kv_a_ln[:, None] * Wkvb).reshape(KV_LORA, H, D_NOPE + D_V)
    wkvb_cores = []
    for c in range(NCORES):
        hs = range(c * HL, (c + 1) * HL)
        cols = [wkvb_eff[:, h_, :D_NOPE] for h_ in hs]
        cols += [wkvb_eff[:, h_, D_NOPE:] for h_ in hs]
        wkvb_cores.append(np.ascontiguousarray(
            np.concatenate(cols, axis=1).astype(BF)))

    # Wo rows permuted to the stage-C gather order: slot j, rank r -> head 4r+j
    row_order = []
    for j in range(HL):
        for r in range(NCORES):
            h_ = HL * r + j
            row_order.extend(range(h_ * D_V, (h_ + 1) * D_V))
    wo_p = Wo[row_order, :].astype(BF)

    # rope ext tiles for q (2 heads per 128-row tile: [e,o | e,o] x 32 rows each)
    cosT = cos.T  # [32, T]
    sinT = sin.T
    ccq = np.ascontiguousarray(np.tile(cosT, (4, 1)).astype(BF))      # [128, T]
    ssq = np.ascontiguousarray(
        np.concatenate([-sinT, sinT, -sinT, sinT], axis=0).astype(BF))

    # swap permutation: within each 64-row block, rows 0:32 <-> 32:64
    pswm = np.zeros((128, 128), dtype=np.float32)
    for j in range(128):
        base = (j // 64) * 64
        off = j % 64
        k = base + (off + HR) % 64
        pswm[k, j] = 1.0
    pswm = pswm.astype(BF)

    # causal masks for the 4 diagonal offsets (512-wide q blocks, 128-wide k chunks)
    pos = positions.astype(np.int64)
    maskd = np.zeros((128, 4 * 512), dtype=np.float32)
    for d in range(4):
        kpos = pos[d * 128:(d + 1) * 128]   # relative within a q block
        qpos = pos[0:512]
        maskd[:, d * 512:(d + 1) * 512] = (kpos[:, None] <= qpos[None, :]).astype(np.float32)
    maskd = maskd.astype(BF)

    per_core = []
    for c in range(NCORES):
        sl = slice(c * TSH, (c + 1) * TSH)
        per_core.append({
            "hT": np.ascontiguousarray(hidden_states[sl].T.astype(BF)),
            "wqa": wqa_b,
            "wkva": wkva_b,
            "wqb": wqb_cores[c],
            "wkvb": wkvb_cores[c],
            "wo": np.ascontiguousarray(wo_p[:, c * WO_N:(c + 1) * WO_N]),
            "cca": np.ascontiguousarray(cosT[:, sl]),
            "ssa": np.ascontiguousarray(sinT[:, sl]),
            "ccq": ccq,
            "ssq": ssq,
            "psw": pswm,
            "maskd": maskd,
            "onesd": np.ones((128, 1), dtype=np.float32).astype(BF),
        })
    return per_core


def run(inputs, trace=False):
    """Build + run; returns (full_output [T, HID] fp32, exec_time_ns or None)."""
    _install_ntff_hook()
    from concourse.bass_utils import run_bass_kernel_spmd

    nc = _build_program()
    in_maps = _prep_inputs(**inputs)
    res = run_bass_kernel_spmd(nc, in_maps, list(range(NCORES)), trace=trace)
    out = np.empty((T, HID), dtype=np.float32)
    for c in range(NCORES):
        out[:, c * WO_N:(c + 1) * WO_N] = res.results[c]["out"].T
    return out, res.exec_time_ns


def kernel(**inputs):
    out, _ = run(inputs, trace=False)
    return out
